# revision 25
# baseline (speedup 1.0000x reference)
"""Trainium2 Bass kernel for ragged clause attention-pooling (BertEncoder head).

Reference computation (per batch element b):
  offsets = exclusive-cumsum(clause_len)            # clause d occupies tokens
  pos[d,c] = offsets[d] + c                         #   [offsets[d], offsets[d]+len[d])
  valid(d,c) = c < clause_len[d] and d < doc_len
  sent[d,c,:] = hidden[pos[d,c],:] * valid
  alpha = sent @ fc_w + fc_b ; masked-softmax over c ; out[d,:] = w @ sent[d]

Key structural facts exploited here:
  * Valid tokens of all clauses tile the contiguous prefix [0, T_b) of the
    token stream, T_b = sum of valid clause lengths (max 2016 < L). Only that
    prefix is ever read.
  * fc_b and the softmax max-subtraction shift alpha uniformly within a
    clause, so softmax weights are exp(score)/segsum with score = hidden@fc_w.
  * out = G.T @ hidden_prefix with G[t,d] = exp(score[t]) * S[t,d], where
    S[t,d] = 1 iff token t belongs to clause d (host-built 0/1 matrix).
    A ones column appended to hidden makes the segment sums ride the same
    matmul (seg[d] = last output column).

Device pipeline per 128-token tile: fused DVE multiply-reduce for score,
ACT exp, GpSimd scale of S^T by p, then PE matmuls with G as the (cheap,
32-col) stationary operand accumulating out[d, h] in PSUM. Epilogue:
reciprocal of segment sums, per-partition scale on the Scalar engine.

Sharding: pure data parallel. Batches are sorted by prefix length and striped
across 8 cores x SPC slots so one shared SPMD program (slot sizes = per-rank-
group maxima) covers every core with minimal padding.

DTYPE: fp16 for the streamed data (hidden/S^T/weights). Error vs the fp32
reference is ~5e-4 relative (fp16 input rounding); PSUM accumulation and the
softmax normalization stay fp32. Set DTYPE="f32" for a bit-accurate (~2e-6)
but ~2x slower variant.
"""

import os
import sys

import numpy as np

for _p in ("/opt/trn_rl_repo",):
    if _p not in sys.path and os.path.isdir(_p):
        sys.path.insert(0, _p)

PART = 128          # SBUF partitions / matmul contraction tile
N_CORES = 8
EPS = 1e-30         # empty-clause guard: 0/(0+EPS) == 0, matching reference
DTYPE = "f16"       # "f16" | "f32" — streamed-data precision

# Exposed for the test harness: BassKernelResults of the most recent run.
LAST_RESULT = None

_PROGRAM_CACHE: dict = {}


def _np_dtype():
    return np.float16 if DTYPE == "f16" else np.float32


def _build_program(nt: tuple, H: int, D: int):
    """One SPMD program processing SPC ragged slots; slot s has nt[s] token
    tiles of 128 resident in SBUF."""
    import concourse.bacc as bacc
    import concourse.mybir as mybir
    import concourse.tile as tile

    f32 = mybir.dt.float32
    dt = mybir.dt.float16 if DTYPE == "f16" else mybir.dt.float32
    SPC = len(nt)
    NH = H // 2                          # PSUM bank limit: N <= 512 fp32 out
    # ones column (seg sums) at H, zero pad at H+1 keeps fp16 rows 4B-aligned
    # so the DVE 16-bit 2x packing mode stays eligible
    HW = H + 2

    nc = bacc.Bacc("TRN2", target_bir_lowering=False, num_devices=N_CORES)

    w_dram = nc.dram_tensor("w128", [PART, H], dt, kind="ExternalInput")
    hs_dram = [
        nc.dram_tensor(f"hs{s}", [PART, nt[s], HW], dt, kind="ExternalInput")
        for s in range(SPC)
    ]
    st_dram = [
        nc.dram_tensor(f"st{s}", [PART, nt[s], D], dt, kind="ExternalInput")
        for s in range(SPC)
    ]
    out_dram = nc.dram_tensor("out", [SPC, D, H], f32, kind="ExternalOutput")

    with tile.TileContext(nc) as tc:
        with (
            tc.tile_pool(name="const", bufs=1) as cpool,
            tc.tile_pool(name="data", bufs=1) as dpool,
            tc.tile_pool(name="small", bufs=4) as spool,
            tc.tile_pool(name="junk", bufs=2) as jpool,
            tc.tile_pool(name="outp", bufs=2) as opool,
            tc.tile_pool(name="psum", bufs=2, space="PSUM") as ppool,
        ):
            w_t = cpool.tile([PART, H], dt, tag="w")
            nc.sync.dma_start(w_t[:], w_dram[:])

            def emit_epilogue(s, oA, oB):
                seg_eps = spool.tile([D, 1], f32, tag="sege", name="sege")
                nc.vector.tensor_scalar_add(seg_eps[:], oB[:, NH : NH + 1], EPS)
                recip = spool.tile([D, 1], f32, tag="recip", name="recip")
                nc.vector.reciprocal(recip[:], seg_eps[:])
                outT = opool.tile([D, H], f32, tag="outT", name="outT")
                nc.scalar.mul(outT[:, 0:NH], oA[:], recip[:])
                nc.scalar.mul(outT[:, NH:H], oB[:, 0:NH], recip[:])
                nc.sync.dma_start(out_dram[s, :, :], outT[:])

            pending = None
            for s in range(SPC):
                n = nt[s]
                hs_t = dpool.tile([PART, n, HW], dt, tag=f"hs{s}")
                st_t = dpool.tile([PART, n, D], dt, tag=f"st{s}")
                # small leading chunks so the first score op starts early
                # (slot 0's first tile lands before st/bulk data)
                chunks, j0 = [], 0
                for sz in ([1, 1, 2] if s == 0 else [4]):
                    if j0 >= n:
                        break
                    chunks.append((j0, min(j0 + sz, n)))
                    j0 = chunks[-1][1]
                while j0 < n:
                    chunks.append((j0, min(j0 + 4, n)))
                    j0 = chunks[-1][1]
                for ci, (a, b) in enumerate(chunks):
                    nc.sync.dma_start(hs_t[:, a:b, :], hs_dram[s][:, a:b, :])
                    if ci == 0:
                        nc.sync.dma_start(st_t[:], st_dram[s][:])

                # G is the (cheap, 32-col) stationary operand; hs streams as
                # the moving operand. out[d, h] accumulates in [D, H(+1)]
                # PSUM; the ones column makes oB[:, NH] the segment sums.
                oA = ppool.tile([D, NH], f32, tag="oA")
                oB = ppool.tile([D, NH + 2], f32, tag="oB")

                for j in range(n):
                    start, stop = (j == 0), (j == n - 1)
                    junk = jpool.tile([PART, H], dt, tag="junk")
                    score = spool.tile([PART, 1], f32, tag="score")
                    # score[t] = sum_h hs[t,h] * w[h] — one fused DVE pass.
                    # (TENSOR_TENSOR_REDUCE wedges this runtime; the
                    # TensorScalarPtr form computes the same thing.)
                    nc.vector.scalar_tensor_tensor(
                        junk[:],
                        hs_t[:, j, 0:H],
                        1.0,
                        w_t[:],
                        mybir.AluOpType.mult,
                        mybir.AluOpType.mult,
                        accum_out=score[:],
                    )
                    p = spool.tile([PART, 1], f32, tag="p")
                    nc.scalar.activation(
                        p[:], score[:], mybir.ActivationFunctionType.Exp
                    )
                    g = spool.tile([PART, D], dt, tag="g")
                    # G[t, d] = S^T[t, d] * p[t] (per-partition scale on the
                    # Scalar engine, right after its exp — no extra handoff)
                    nc.scalar.mul(g[:], st_t[:, j, :], p[:])
                    # out[d, h] += g[t, d] * hs[t, h]; ones col -> seg sums
                    nc.tensor.matmul(
                        oA[:], g[:], hs_t[:, j, 0:NH], start=start, stop=stop
                    )
                    nc.tensor.matmul(
                        oB[:], g[:], hs_t[:, j, NH:HW], start=start, stop=stop
                    )

                # defer this slot's epilogue until after the next slot's
                # tile loop: keeps the DVE queue free of ops that wait on
                # this slot's final matmul (no slot-boundary stalls)
                if pending is not None:
                    emit_epilogue(*pending)
                pending = (s, oA, oB)

            emit_epilogue(*pending)

    nc.compile()
    in_names = (
        ["w128"]
        + [f"hs{s}" for s in range(SPC)]
        + [f"st{s}" for s in range(SPC)]
    )
    return nc, in_names


def _ensure_axon_hooks():
    """concourse.bass_utils' trace path does an unguarded import of
    antenv.axon_hooks; some images lack that module. Provide a registry that
    builds the ctypes NTFF hook on demand (or degrades to no tracing)."""
    try:
        import antenv.axon_hooks  # noqa: F401

        return
    except Exception:
        pass
    import types

    mod = types.ModuleType("antenv.axon_hooks")
    mod._NTFF_PROFILE_HOOK = None

    def set_axon_ntff_profile_hook(hook):
        mod._NTFF_PROFILE_HOOK = hook

    def get_axon_ntff_profile_hook():
        if mod._NTFF_PROFILE_HOOK is None:
            try:
                from trn_agent_boot.trn_boot import _ntff_profile_via_ctypes

                mod._NTFF_PROFILE_HOOK = _ntff_profile_via_ctypes(
                    "/opt/axon/libaxon_pjrt.so"
                )
            except Exception:
                return None
        return mod._NTFF_PROFILE_HOOK

    mod.set_axon_ntff_profile_hook = set_axon_ntff_profile_hook
    mod.get_axon_ntff_profile_hook = get_axon_ntff_profile_hook
    sys.modules["antenv.axon_hooks"] = mod
    try:
        import antenv

        antenv.axon_hooks = mod
    except Exception:
        pass


def _prep_core_inputs(hs, w, cl, dl, offs, T, order, nt, core, H, D):
    """Build one core's input map: fp16/fp32 padded prefixes with a ones
    column, the clause-indicator S^T, and the replicated fc weights."""
    npdt = _np_dtype()
    SPC = len(nt)
    in_map = {
        "w128": np.ascontiguousarray(
            np.broadcast_to(w.astype(npdt), (PART, H))
        )
    }
    for s in range(SPC):
        P = nt[s] * PART
        b = int(order[s * N_CORES + core])
        t = int(T[b])
        hp = np.zeros((P, H + 2), npdt)
        hp[:t, :H] = hs[b, :t].astype(npdt)
        hp[:, H] = 1
        st = np.zeros((P, D), npdt)
        for d in range(int(dl[b])):
            ln = int(cl[b, d])
            if ln > 0:
                o = int(offs[b, d])
                st[o : o + ln, d] = 1
        in_map[f"hs{s}"] = hp.reshape(PART, nt[s], H + 2)
        in_map[f"st{s}"] = st.reshape(PART, nt[s], D)
    return in_map


def kernel(hidden_states, fc_w, fc_b, clause_len, doc_len):
    global LAST_RESULT
    _ensure_axon_hooks()
    from concourse.bass_utils import run_bass_kernel_spmd

    hs = np.ascontiguousarray(np.asarray(hidden_states, dtype=np.float32))
    w = np.asarray(fc_w, dtype=np.float32).reshape(-1)
    cl = np.asarray(clause_len).astype(np.int64)
    dl = np.asarray(doc_len).astype(np.int64).reshape(-1)
    B, L, H = hs.shape
    D = cl.shape[1]
    assert B % N_CORES == 0 and H % PART == 0
    SPC = B // N_CORES

    offs = np.cumsum(cl, axis=1) - cl                       # [B, D]
    # T_b: tokens used by valid clauses (clauses tile the prefix contiguously)
    T = np.zeros(B, dtype=np.int64)
    for b in range(B):
        d = int(dl[b])
        if d > 0:
            T[b] = int(offs[b, d - 1] + cl[b, d - 1])
    T = np.minimum(T, L)

    order = np.argsort(-T, kind="stable")                   # rank -> batch
    nt = tuple(
        max(1, -(-int(T[order[s * N_CORES : (s + 1) * N_CORES]].max()) // PART))
        for s in range(SPC)
    )

    key = (nt, B, L, H, D, DTYPE)
    if key not in _PROGRAM_CACHE:
        _PROGRAM_CACHE[key] = _build_program(nt, H, D)
    nc, in_names = _PROGRAM_CACHE[key]

    in_maps = [
        _prep_core_inputs(hs, w, cl, dl, offs, T, order, nt, c, H, D)
        for c in range(N_CORES)
    ]

    res = run_bass_kernel_spmd(nc, in_maps, core_ids=list(range(N_CORES)))
    LAST_RESULT = res

    out = np.zeros((B, D, H), np.float32)
    for c in range(N_CORES):
        a = np.asarray(res.results[c]["out"])               # [SPC, D, H]
        for s in range(SPC):
            out[int(order[s * N_CORES + c])] = a[s]
    return out
